# revision 17
# baseline (speedup 1.0000x reference)
"""Trainium2 Bass kernel for nn_DownSample2 (retrieval_knn).

Self-contained: builds the Bass program, shards bs=16 across 8 NeuronCores
(2 samples per core), runs via run_bass_kernel_spmd, gathers full outputs.

Reference computation per sample:
  fps_idx = FPS(stk_fea^T, 16); knn_idx = top2-NN of stk_fea; nn = knn[:,1]
  sf  = gelu(BN(sp_w @ max(sparse[:,fps], sparse[:,nn[fps]])))      [1024,16]
  df  = gather+pairmax over strokes of gelu(BN(conv1x3s2(dense)))   [1024,512]
  stk = stk_fea[:, fps]                                             [512,16]
"""

import numpy as np

import concourse.bass as bass
import concourse.tile as tile
from concourse import bacc, mybir

BS = 16
EMB = 512
N_STK = 32
N_PNT = 2048
PPAD = 66      # padded points per stroke (zero at cols 0 and 65)
NS2 = 16          # N_STK // 2 sampled strokes
PTS = 64          # points per stroke
PTS2 = 32         # after stride-2 conv
OUT_CH = 1024
EPS = 1e-5
S = 2             # samples per core
NCORES = 8

F32 = mybir.dt.float32
F32R = mybir.dt.float32r
U32 = mybir.dt.uint32
DVE = mybir.EngineType.DVE
AX = mybir.AxisListType.X
ALU = mybir.AluOpType
AF = mybir.ActivationFunctionType


def emit(nc, tc, t, conv_dtype=F32R):
    """Emit the per-core program. t: dict name->AP of dram tensors."""
    import contextlib
    ctx = contextlib.ExitStack()
    with ctx:
        # ------------- pools -------------
        cw = ctx.enter_context(tc.tile_pool(name="cw", bufs=1))       # weights/consts
        sm = ctx.enter_context(tc.tile_pool(name="sm", bufs=1))       # small persistents
        dp = ctx.enter_context(tc.tile_pool(name="dp", bufs=2))       # dense chunks
        osp = ctx.enter_context(tc.tile_pool(name="osp", bufs=1))     # shifted-odd tiles
        dfp = ctx.enter_context(tc.tile_pool(name="dfp", bufs=1))     # DF chunks
        selp = ctx.enter_context(tc.tile_pool(name="selp", bufs=1))   # df gather out
        gp = ctx.enter_context(tc.tile_pool(name="gp", bufs=1))       # gathered DF
        ps_small = ctx.enter_context(tc.tile_pool(name="pss", bufs=2, space="PSUM"))
        ps_conv = ctx.enter_context(tc.tile_pool(name="psc", bufs=4, space="PSUM"))
        ps_sp = ctx.enter_context(tc.tile_pool(name="pssp", bufs=2, space="PSUM"))

        # ------------- consts & weights into SBUF -------------
        def load(pool, name, shape, dt, tag=None):
            tl = pool.tile(shape, dt, tag=tag or name, name=name)
            nc.sync.dma_start(tl[:], t[name][:])
            return tl

        id128 = load(cw, "id128", [128, 128], F32)
        id16 = load(cw, "id16", [16, 16], F32)
        p32f = load(cw, "p32f", [32, 1], F32)
        ones32 = load(cw, "ones32", [1, 32], F32)
        blk2_64 = load(cw, "blk2_64", [2, 64], F32)
        cst010 = load(cw, "cst010", [128, 3], F32)
        dnw = load(cw, "dnw", [128, 12 * 1024], conv_dtype)
        spw = load(cw, "spw", [128, 4 * 1024], F32)
        dnb = load(cw, "dnb", [128, 8], F32)
        spb = load(cw, "spb", [128, 8], F32)
        zu2 = load(cw, "zu2", [1, 2], U32)

        # stk / sparse chunks  [128, 32] x (s, k)
        stkc = [[None] * 4 for _ in range(S)]
        spc = [[None] * 4 for _ in range(S)]
        for s in range(S):
            for k in range(4):
                stkc[s][k] = sm.tile([128, 32], F32, tag=f"stk{s}{k}", name=f"stk{s}{k}")
                nc.sync.dma_start(stkc[s][k][:], t["stk"][s, 128 * k:128 * (k + 1), :])
                spc[s][k] = sm.tile([128, 32], F32, tag=f"spc{s}{k}", name=f"spc{s}{k}")
                nc.sync.dma_start(spc[s][k][:], t["sparse"][s, 128 * k:128 * (k + 1), :])

        # ------------- stage A: KNN + FPS -------------
        # squares of stk
        x2 = [[None] * 4 for _ in range(S)]
        for s in range(S):
            for k in range(4):
                x2[s][k] = sm.tile([128, 32], F32, tag=f"x2{s}{k}", name=f"x2{s}{k}")
                nc.scalar.square(x2[s][k][:], stkc[s][k][:])

        # Gram matrices per sample -> SBUF [32, 32]
        g_sb = []
        for s in range(S):
            g_ps = ps_small.tile([32, 32], F32, tag="small", name=f"g_ps{s}")
            for k in range(4):
                nc.tensor.matmul(g_ps[:], stkc[s][k][:], stkc[s][k][:],
                                 start=(k == 0), stop=(k == 3))
            g = sm.tile([32, 32], F32, tag=f"g_sb{s}", name=f"g_sb{s}")
            nc.vector.tensor_copy(g[:], g_ps[:])
            g_sb.append(g)

        # xx rows [2, 32]
        xx_ps = ps_small.tile([2, 32], F32, tag="small", name="xx_ps")
        first = True
        for s in range(S):
            lhs = cst010[:, 1:3] if s == 0 else cst010[:, 0:2]
            for k in range(4):
                nc.tensor.matmul(xx_ps[:], lhs, x2[s][k][:],
                                 start=first, stop=(s == S - 1 and k == 3))
                first = False
        xx_sb = sm.tile([2, 32], F32, tag="xx_sb")
        nc.vector.tensor_copy(xx_sb[:], xx_ps[:])

        # per-sample processing: broadcast xx, pd2/KNN, D
        ix8k = []
        d_cols = []
        for s in range(S):
            xxb_ps = ps_small.tile([32, 32], F32, tag="small", name=f"xxb_ps{s}")
            nc.tensor.matmul(xxb_ps[:], blk2_64[:, 32 * s:32 * (s + 1)], xx_sb[:],
                             start=True, stop=True)
            xxb = sm.tile([32, 32], F32, tag=f"xxb{s}", name=f"xxb{s}")
            nc.vector.tensor_copy(xxb[:], xxb_ps[:])

            # pd2 = 2G - xx[m]  -> KNN
            pd2 = sm.tile([32, 32], F32, tag=f"pd2{s}", name=f"pd2{s}")
            nc.vector.scalar_tensor_tensor(pd2[:], g_sb[s][:], 2.0, xxb[:],
                                           op0=ALU.mult, op1=ALU.subtract)
            mx8k = sm.tile([32, 8], F32, tag=f"mx8k{s}", name=f"mx8k{s}")
            ixk = sm.tile([32, 8], U32, tag=f"ix8k{s}", name=f"ix8k{s}")
            nc.vector.max(mx8k[:], pd2[:])
            nc.vector.max_index(ixk[:], mx8k[:], pd2[:])
            ix8k.append(ixk)

            # full distance D = xx[n] + xx[m] - 2G   [32, 32]
            xxt_m = sm.tile([32, 32], F32, tag=f"xxt_m{s}", name=f"xxt_m{s}")
            nc.vector.tensor_tensor(xxt_m[:], g_sb[s][:], id128[0:32, 0:32], ALU.mult)
            xxt = sm.tile([32, 1], F32, tag=f"xxt{s}", name=f"xxt{s}")
            nc.vector.reduce_sum(xxt[:], xxt_m[:], axis=AX)
            d_col = sm.tile([32, 32], F32, tag=f"d_col{s}", name=f"d_col{s}")
            nc.vector.scalar_tensor_tensor(d_col[:], g_sb[s][:], -2.0, xxb[:],
                                           op0=ALU.mult, op1=ALU.add)
            nc.vector.tensor_scalar(d_col[:], d_col[:], xxt[:], None, op0=ALU.add)
            d_cols.append(d_col)

        # FPS loop.  dist row on partition 0: [s0 cols 0:32 | s1 32:64].
        # Row selection D[far] via one-hot matmul (D symmetric -> use columns).
        fps_sb = sm.tile([1, 32], U32, tag="fps_sb")    # cols 16*s + j
        nc.vector.tensor_copy(fps_sb[:, 0:1], zu2[:, 0:1])
        nc.vector.tensor_copy(fps_sb[:, 16:17], zu2[:, 1:2])
        dist = sm.tile([1, 64], F32, tag="dist")
        for s in range(S):
            nc.vector.tensor_copy(dist[:, 32 * s:32 * (s + 1)], d_cols[s][0:1, :])
        mx8 = sm.tile([1, 8], F32, tag="mx8")
        ix8 = sm.tile([1, 16], U32, tag="ix8")          # cols 8*s
        for j in range(1, NS2):
            for s in range(S):
                nc.vector.max(mx8[:], dist[:, 32 * s:32 * (s + 1)])
                nc.vector.max_index(ix8[:, 8 * s:8 * (s + 1)], mx8[:],
                                    dist[:, 32 * s:32 * (s + 1)])
                nc.vector.tensor_copy(fps_sb[:, 16 * s + j:16 * s + j + 1],
                                      ix8[:, 8 * s:8 * s + 1])
            if j < NS2 - 1:
                far_f = sm.tile([1, 2], F32, tag="far_f")
                nc.vector.tensor_copy(far_f[:], ix8[0:1, 0:9:8])
                farb = ps_small.tile([32, 2], F32, tag="small", name=f"farb_{j}")
                nc.tensor.matmul(farb[:], ones32[:], far_f[:], start=True, stop=True)
                oh2 = sm.tile([32, 2], F32, tag="oh2")
                nc.vector.tensor_scalar(oh2[:], farb[:], p32f[:], None, op0=ALU.is_equal)
                rsel = ps_small.tile([1, 64], F32, tag="small", name=f"rsel_{j}")
                for s in range(S):
                    nc.tensor.matmul(rsel[0:1, 32 * s:32 * (s + 1)], oh2[:, s:s + 1],
                                     d_cols[s][:], start=True, stop=True)
                nc.vector.tensor_tensor(dist[:], dist[:], rsel[:], ALU.min)

        # ------------- one-hot matrices -------------
        # oh cols: [fps (16s+j) | nn interleaved (32 + 2j + s)]
        fps_f32 = sm.tile([1, 32], F32, tag="fps_f32")
        nc.vector.tensor_copy(fps_f32[:], fps_sb[:])
        oh = sm.tile([32, 64], F32, tag="oh")
        farb1 = ps_small.tile([32, 32], F32, tag="small", name="farb1")
        nc.tensor.matmul(farb1[:], ones32[:], fps_f32[:], start=True, stop=True)
        nc.vector.tensor_scalar(oh[:, 0:32], farb1[:], p32f[:], None, op0=ALU.is_equal)

        # nn gathered at fps: nnf[j] = nn[fps[j]]
        nnf_ps = ps_small.tile([16, 2], F32, tag="small", name="nnf_ps")
        for s in range(S):
            nn_f32 = sm.tile([32, 1], F32, tag=f"nn_f32{s}", name=f"nn_f32{s}")
            nc.vector.tensor_copy(nn_f32[:], ix8k[s][:, 1:2])
            nc.tensor.matmul(nnf_ps[:, s:s + 1], oh[:, 16 * s:16 * (s + 1)],
                             nn_f32[:], start=True, stop=True)
        nnf_sb = sm.tile([16, 2], F32, tag="nnf_sb")
        nc.vector.tensor_copy(nnf_sb[:], nnf_ps[:])
        # [16, 2] -> row [1, 32] (interleaved cols 2j+s) via sbuf-to-sbuf dma
        nnf_row = sm.tile([1, 32], F32, tag="nnf_row")
        nc.sync.dma_start(nnf_row[:], nnf_sb[:])
        farb2 = ps_small.tile([32, 32], F32, tag="small", name="farb2")
        nc.tensor.matmul(farb2[:], ones32[:], nnf_row[:], start=True, stop=True)
        nc.vector.tensor_scalar(oh[:, 32:64], farb2[:], p32f[:], None, op0=ALU.is_equal)

        # int16 gather-index tiles for ap_gather: [128, 2] per sample,
        # col0 = fps_j at partition j%16, col1 = nnf_j; replicated per 16 rows.
        idx128 = []
        for s in range(S):
            idxp = ps_small.tile([16, 2], F32, tag="small", name=f"idxp{s}")
            nc.tensor.transpose(idxp[:, 0:1], fps_f32[0:1, 16 * s:16 * (s + 1)],
                                ones32[0:1, 0:1])
            nc.tensor.transpose(idxp[:, 1:2], nnf_row[0:1, s:32:2], ones32[0:1, 0:1])
            idx16 = sm.tile([16, 2], mybir.dt.int16, tag=f"idx16_{s}", name=f"idx16_{s}")
            nc.vector.tensor_copy(idx16[:], idxp[:])
            rep = sm.tile([128, 2], mybir.dt.int16, tag=f"idx128_{s}", name=f"idx128_{s}")
            for gpg in range(8):
                nc.sync.dma_start(rep[16 * gpg:16 * (gpg + 1), :], idx16[:])
            idx128.append(rep)

        # ------------- stage B: sparse & stk branches -------------
        oh_nn3 = oh[:, 32:64].rearrange("p (j s) -> p j s", s=2)
        sfmax = sm.tile([128, 4, 2, 16], F32, tag="sfmax")  # (k, s, j)
        for s in range(S):
            for k in range(4):
                # transpose chunks
                tp = ps_small.tile([32, 128], F32, tag="small", name="tp")
                nc.tensor.transpose(tp[:], spc[s][k][:], id128[:])
                spt = sm.tile([32, 128], F32, tag=f"spt{s}{k}")
                nc.vector.tensor_copy(spt[:], tp[:])
                tp2 = ps_small.tile([32, 128], F32, tag="small", name="tp2")
                nc.tensor.transpose(tp2[:], stkc[s][k][:], id128[:])
                stt = sm.tile([32, 128], F32, tag=f"stt{s}{k}")
                nc.vector.tensor_copy(stt[:], tp2[:])

                # sparse gather (both fps and nn in one matmul)
                g01 = ps_small.tile([128, 32], F32, tag="small", name="g01")
                nc.tensor.matmul(g01[:, 0:16], spt[:], oh[:, 16 * s:16 * (s + 1)],
                                 start=True, stop=True)
                nc.tensor.matmul(g01[:, 16:32], spt[:], oh_nn3[:, :, s],
                                 start=True, stop=True)
                g0c = sm.tile([128, 16], F32, tag="g0c")
                nc.vector.tensor_copy(g0c[:], g01[:, 0:16])
                nc.vector.tensor_tensor(sfmax[:, k, s, :], g0c[:], g01[:, 16:32],
                                        ALU.max)

                # stk gather -> output
                stg = ps_small.tile([128, 16], F32, tag="small", name="stg")
                nc.tensor.matmul(stg[:], stt[:], oh[:, 16 * s:16 * (s + 1)],
                                 start=True, stop=True)
                stgs = sm.tile([128, 16], F32, tag="stgs")
                nc.vector.tensor_copy(stgs[:], stg[:])
                nc.sync.dma_start(t["stk_s"][s, 128 * k:128 * (k + 1), :], stgs[:])

        # sp conv (both samples at once)
        for m in range(8):
            pssp = ps_sp.tile([128, 32], F32, tag="pssp")
            for k in range(4):
                nc.tensor.matmul(pssp[:], spw[:, 1024 * k + 128 * m:1024 * k + 128 * (m + 1)],
                                 sfmax[:, k, :, :], start=(k == 0), stop=(k == 3))
            sf_sb = sm.tile([128, 32], F32, tag="sf_sb")
            nc.scalar.activation(sf_sb[:], pssp[:], AF.Gelu, bias=spb[:, m:m + 1])
            for s in range(S):
                nc.sync.dma_start(t["sf"][s, 128 * m:128 * (m + 1), :],
                                  sf_sb[:, 16 * s:16 * (s + 1)])

        # ------------- stage C: dense conv + gather -------------
        for s in range(S):
            dch = []
            for k in range(4):
                d_t = dp.tile([128, N_STK * PPAD], conv_dtype, tag=f"dense{k}", name=f"dense{k}")
                nc.sync.dma_start(d_t[:], t["dense"][s, 128 * k:128 * (k + 1), :])
                dch.append(d_t)

            for grp in range(2):       # two groups of 4 m-chunks
                # DF layout: [128, 32 strokes, 4 m, 32 pts] (stroke-major for gather)
                df_t = dfp.tile([128, N_STK, 4, PTS2], F32, tag="df")
                df4 = df_t[:]
                for mi in range(4):
                    m = grp * 4 + mi
                    for nc2 in range(2):
                        psc = ps_conv.tile([128, 512], F32, tag="psc")
                        d3s = [dch[k][:].rearrange("p (sg q) -> p sg q", q=PPAD)
                               [:, 16 * nc2:16 * (nc2 + 1), :] for k in range(4)]
                        nmm = 0
                        for tap, off in ((1, 1), (2, 2), (0, 0)):
                            for k in range(4):
                                lo = (tap * 4 + k) * 1024 + 128 * m
                                rhs = d3s[k][:, :, off:off + 63:2]
                                nc.tensor.matmul(psc[:], dnw[:, lo:lo + 128], rhs,
                                                 start=(nmm == 0), stop=(nmm == 11))
                                nmm += 1
                        nc.scalar.activation(df4[:, 16 * nc2:16 * (nc2 + 1), mi, :],
                                             psc[:], AF.Gelu, bias=dnb[:, m:m + 1])

                g_t = gp.tile([128, N_STK, 4 * PTS2], F32, tag="g")
                nc.gpsimd.ap_gather(g_t[:], df4.rearrange("p a b c -> p a (b c)"),
                                    idx128[s][:], channels=128, num_elems=N_STK,
                                    d=4 * PTS2, num_idxs=32)
                sel = selp.tile([128, NS2, 4, PTS2], F32, tag="sel")
                nc.vector.tensor_tensor(sel[:].rearrange("p a b c -> p a (b c)"),
                                        g_t[:, 0:16, :], g_t[:, 16:32, :], ALU.max)
                for mi in range(4):
                    m = grp * 4 + mi
                    nc.sync.dma_start(t["df"][s, 128 * m:128 * (m + 1), :],
                                      sel[:, :, mi, :])


def build_nc(num_devices=NCORES, conv_dtype=F32R):
    nc = bacc.Bacc("TRN2", target_bir_lowering=False, debug=False,
                   num_devices=num_devices)
    t = {}

    def dram(name, shape, dt, kind):
        t[name] = nc.dram_tensor(name, shape, dt, kind=kind).ap()

    dram("dense", [S, EMB, N_STK * PPAD], conv_dtype, "ExternalInput")
    dram("sparse", [S, EMB, N_STK], F32, "ExternalInput")
    dram("stk", [S, EMB, N_STK], F32, "ExternalInput")
    dram("dnw", [128, 12 * 1024], conv_dtype, "ExternalInput")
    dram("spw", [128, 4 * 1024], F32, "ExternalInput")
    dram("dnb", [128, 8], F32, "ExternalInput")
    dram("spb", [128, 8], F32, "ExternalInput")
    dram("zu2", [1, 2], U32, "ExternalInput")
    dram("id128", [128, 128], F32, "ExternalInput")
    dram("id16", [16, 16], F32, "ExternalInput")
    dram("p32f", [32, 1], F32, "ExternalInput")
    dram("ones32", [1, 32], F32, "ExternalInput")
    dram("blk2_64", [2, 64], F32, "ExternalInput")
    dram("cst010", [128, 3], F32, "ExternalInput")
    dram("sf", [S, OUT_CH, NS2], F32, "ExternalOutput")
    dram("df", [S, OUT_CH, 512], F32, "ExternalOutput")
    dram("stk_s", [S, EMB, NS2], F32, "ExternalOutput")

    with tile.TileContext(nc) as tc:
        emit(nc, tc, t, conv_dtype=conv_dtype)
    nc.compile()
    return nc


def host_weights(inputs):
    """Fold BN into conv weights; build lhsT layouts and constants."""
    o = {}
    sc = inputs["dn_g"] / np.sqrt(inputs["dn_v"] + EPS)
    w = inputs["dn_w"][:, :, 0, :] * sc[:, None, None]       # [1024, 512, 3]
    bb = (inputs["dn_b"] - inputs["dn_m"]) * sc + inputs["dn_be"]
    dnw = np.empty((128, 12 * 1024), np.float32)
    for tap in range(3):
        for k in range(4):
            blk = w[:, 128 * k:128 * (k + 1), tap].T         # [128, 1024]
            dnw[:, (tap * 4 + k) * 1024:(tap * 4 + k + 1) * 1024] = blk
    o["dnw"] = dnw
    o["dnb"] = np.ascontiguousarray(bb.reshape(8, 128).T)    # [128, 8]

    sc2 = inputs["sp_g"] / np.sqrt(inputs["sp_v"] + EPS)
    w2 = inputs["sp_w"] * sc2[:, None]
    bb2 = (inputs["sp_b"] - inputs["sp_m"]) * sc2 + inputs["sp_be"]
    spw = np.empty((128, 4 * 1024), np.float32)
    for k in range(4):
        spw[:, k * 1024:(k + 1) * 1024] = w2[:, 128 * k:128 * (k + 1)].T
    o["spw"] = spw
    o["spb"] = np.ascontiguousarray(bb2.reshape(8, 128).T)
    o["zu2"] = np.zeros((1, 2), np.uint32)

    o["id128"] = np.eye(128, dtype=np.float32)
    o["id16"] = np.eye(16, dtype=np.float32)
    o["p32f"] = np.arange(32, dtype=np.float32).reshape(32, 1)
    o["ones32"] = np.ones((1, 32), np.float32)
    blk = np.zeros((2, 64), np.float32)
    blk[0, 0:32] = 1.0
    blk[1, 32:64] = 1.0
    o["blk2_64"] = blk
    c = np.zeros((128, 3), np.float32)
    c[:, 1] = 1.0
    o["cst010"] = c
    return o


def make_in_maps(inputs):
    wts = host_weights(inputs)
    in_maps = []
    for c in range(NCORES):
        m = dict(wts)
        sl = slice(S * c, S * (c + 1))
        d = inputs["dense_fea"][sl].reshape(S, EMB, N_STK, 64)
        dp_ = np.zeros((S, EMB, N_STK, PPAD), np.float32)
        dp_[:, :, :, 1:65] = d
        m["dense"] = dp_.reshape(S, EMB, N_STK * PPAD)
        m["sparse"] = np.ascontiguousarray(inputs["sparse_fea"][sl])
        m["stk"] = np.ascontiguousarray(inputs["stk_fea"][sl])
        in_maps.append(m)
    return in_maps


_NC_CACHE = {}


def get_nc(conv_dtype=F32R):
    key = str(conv_dtype)
    if key not in _NC_CACHE:
        _NC_CACHE[key] = build_nc(conv_dtype=conv_dtype)
    return _NC_CACHE[key]


def kernel(**inputs):
    from concourse import bass_utils
    inputs = {k: np.asarray(v) for k, v in inputs.items()}
    nc = get_nc()
    in_maps = make_in_maps(inputs)
    res = bass_utils.run_bass_kernel_spmd(nc, in_maps, core_ids=list(range(NCORES)))
    sf = np.concatenate([res.results[c]["sf"] for c in range(NCORES)], axis=0)
    df = np.concatenate([res.results[c]["df"] for c in range(NCORES)], axis=0)
    stk = np.concatenate([res.results[c]["stk_s"] for c in range(NCORES)], axis=0)
    return sf, df, stk
